# revision 17
# baseline (speedup 1.0000x reference)
"""Trainium2 Bass kernel for nn_MessagePassingNN (gnn_message_passing).

B, N, F, H, A, T = 4, 256, 64, 256, 16, 3

Sharding: 8 cores = (batch b, receiver-half). Core c handles batch c//2 and
receiver nodes [128*(c%2), 128*(c%2+1)). All node indexing inside the kernel is
core-RELATIVE ([my 128 | partner 128]) so the SPMD program is identical on all
cores; the host permutes the inputs per core.

Math (per message-passing iteration):
    e[i,j,:] = relu(hi[i,:] + hj[j,:] + b1)           (hi = h@W1_i, hj = h@W1_j)
    agg[i,:] = sum_j adj[i,j] * e[i,j,:] @ W2 + deg[i]*b2     <- linearity trick:
        the W2 matmul is pulled OUT of the j-sum (34 GFLOP -> 134 MFLOP).
    GRU update on agg/h.

Device layout: [feature-on-partitions, node-on-free], e-path in bf16.
E-loop unit = (receiver i, h-tile ht), [128, N] elements. Two lanes:
  - fused DVE lane: one custom DVE op per unit computes
    relu(hjbT + adj_bc + hi) with fp32 accumulation over senders
    (adj_bc holds 32*(adj-1) in {-32,0}, so masked entries die in the relu).
  - ScalarE lane (SC_OF_16 units per 16): mask-inject TT w = hjbT + adj_bc
    (on GpSimd for ht=0, DVE for ht=1), then ACT Relu+bias(hi)+accum.

Iteration tail is receiver-group pipelined (NG groups): each group's
msgT/GRU runs as soon as its aggT columns are done, its pairwise
AllReduce(hnew_g) overlaps the next group's e-loop/GRU; hiT(t+1) and the
local half of hjbT(t+1) are computed under the last group's collective.
Partner hjbT(t+1) columns come from rem=AllReduce result directly:
W1j.T@rem - W1j.T@hnew (negated-weight matmul), skipping hT partner
materialization. The last iteration exchanges only the [2H,1] readout sum.
"""

import sys

sys.path.insert(0, "/opt/trn_rl_repo")

import numpy as np

import concourse.bass as bass
import concourse.bacc as bacc
import concourse.tile as tile
from concourse import mybir
from concourse.bass_utils import run_bass_kernel_spmd

# ---- custom fused DVE op: out = relu(in0 + in1 + s0); accum_out = s1 + sum --
import concourse.dve_ops as dve_ops
from concourse.dve_ops import DveOp
from concourse.dve_spec import Spec, Src0, Src1, C0, C1, relu as _sp_relu, lower
from concourse.dve_spec import AluOp as _SpAluOp
from concourse.dve_uop import DveOpSpec


def _ref_mra(in0, in1, s0, s1, imm2):
    b = np.maximum(in0.astype(np.float32) + in1 + s0, 0.0).astype(np.float32)
    return b, s1 + b.reshape(b.shape[0], -1).sum(axis=-1, keepdims=True)


def _register_fused_op():
    name = "MSG_RELU_ACC_ANT"
    if name in dve_ops._SUB_OPCODE_FOR_NAME:
        return next(o for o in dve_ops.OPS if o.name == name)
    spec = Spec(
        body=_sp_relu(Src0 + Src1 + C0),
        accum=_SpAluOp.ADD,
        accum_init=C1,
        reference=_ref_mra,
    )
    row = max(dve_ops._SUB_OPCODE_FOR_NAME.values()) + 1
    assert row < 0x20
    dve_ops._SUB_OPCODE_FOR_NAME[name] = row
    shas = {}
    for ver in ("v3", "v4"):
        tmp = DveOpSpec(name=name, opcode=row, uops=lower(spec, ver=ver), rd1_en=True)
        shas[ver] = tmp.sha(ver)
    op = DveOp(name, spec, subdim=False, uops_sha=shas)
    dve_ops.OPS.append(op)
    dve_ops.CUSTOM_DVE_SPECS[name] = spec
    return op


MSG_RELU_ACC = _register_fused_op()

B, N, F, H, A, T = 4, 256, 64, 256, 16, 3
NLOC = 128          # receivers per core
HT = H // 128       # h-dim tiles (2)
f32 = mybir.dt.float32
bf16 = mybir.dt.bfloat16
BF16_NP = mybir.dt.np(bf16)

SC_OF_16 = 7        # ScalarE-lane units per 16 (per ht)
MASK_GPS_HT = ()    # mask-inject TTs for these ht run on GpSimd (rest on DVE)
NG = 2              # receiver groups per iteration (cc pipelining)
GW = NLOC // NG     # group width (64)
CH = 8              # adjacency broadcast chunks (16 receivers each)

_CACHE = {}


def _mm_acc(nc, ps, w_sb, m_off, rhs_tiles, kt_count, extra=None):
    """psum ps[:, :] = sum_kt W[kt, m_off:m_off+mw].T @ rhs_tiles[kt]; extra =
    optional (lhsT, rhs) accumulated at the end."""
    n_ins = kt_count + (1 if extra is not None else 0)
    idx = 0
    for kt in range(kt_count):
        lhsT = w_sb[:, kt * w_sb.mcols + m_off: kt * w_sb.mcols + m_off + ps.shape[0]]
        nc.tensor.matmul(ps, lhsT, rhs_tiles[kt], start=(idx == 0), stop=(idx == n_ins - 1))
        idx += 1
    if extra is not None:
        lhsT, rhs = extra
        nc.tensor.matmul(ps, lhsT, rhs, start=False, stop=True)


class _WSb:
    """SBUF weight holder: W [K, M] stored as [128, (K//128)*M]."""

    def __init__(self, nc, pool, dram, K, M, name, queue=None):
        self.mcols = M
        self.kt = K // 128
        self.sb = pool.tile([128, self.kt * M], f32, name=name, tag=name)
        q = queue if queue is not None else nc.sync
        for kt in range(self.kt):
            q.dma_start(
                out=self.sb[:, kt * M:(kt + 1) * M],
                in_=dram[kt * 128:(kt + 1) * 128, :],
            )

    def __getitem__(self, sl):
        return self.sb[sl]


def build_program():
    nc = bacc.Bacc("TRN2", target_bir_lowering=False, debug=False, num_devices=8)

    # ---------------- I/O ----------------
    xT_d = nc.dram_tensor("xT", [F, N], f32, kind="ExternalInput")
    adj_d = nc.dram_tensor("adjb", [NLOC, N], bf16, kind="ExternalInput")
    w_pre1 = nc.dram_tensor("pre_W1", [F, H], f32, kind="ExternalInput")
    w_pre2 = nc.dram_tensor("pre_W2", [H, H], f32, kind="ExternalInput")
    w_m1i = nc.dram_tensor("W1i", [H, H], f32, kind="ExternalInput")
    w_m1j = nc.dram_tensor("W1j", [H, H], f32, kind="ExternalInput")
    w_m1jn = nc.dram_tensor("W1jn", [H, H], f32, kind="ExternalInput")
    w_m2 = nc.dram_tensor("W2m", [H, H], f32, kind="ExternalInput")
    w_ih = nc.dram_tensor("Wih", [H, 3 * H], f32, kind="ExternalInput")
    w_hh = nc.dram_tensor("Whh", [H, 3 * H], f32, kind="ExternalInput")
    w_ro1 = nc.dram_tensor("roW1", [H, H], f32, kind="ExternalInput")
    w_ro2 = nc.dram_tensor("roW2", [H, A], f32, kind="ExternalInput")
    preb1_d = nc.dram_tensor("preb1c", [128, HT], f32, kind="ExternalInput")
    preb2_d = nc.dram_tensor("preb2c", [128, HT], f32, kind="ExternalInput")
    msgb1_d = nc.dram_tensor("msgb1c", [128, HT], f32, kind="ExternalInput")
    msgb2_d = nc.dram_tensor("msgb2r", [1, H], f32, kind="ExternalInput")
    brz_d = nc.dram_tensor("brzc", [128, 4], f32, kind="ExternalInput")
    bihn_d = nc.dram_tensor("bihnc", [128, HT], f32, kind="ExternalInput")
    bhhn_d = nc.dram_tensor("bhhnc", [128, HT], f32, kind="ExternalInput")
    rob1_d = nc.dram_tensor("rob1c", [128, HT], f32, kind="ExternalInput")
    rob2_d = nc.dram_tensor("rob2c", [A, 1], f32, kind="ExternalInput")
    ident_d = nc.dram_tensor("ident", [128, 128], f32, kind="ExternalInput")
    q_out = nc.dram_tensor("q_out", [A, 1], f32, kind="ExternalOutput")

    # collective bounce buffers: per (iteration, group) h-exchange + readout
    cc_in = [[nc.dram_tensor(f"cc_in_{t}_{g}", [H, GW], f32) for g in range(NG)]
             for t in range(T - 1)]
    cc_out = [[nc.dram_tensor(f"cc_out_{t}_{g}", [H, GW], f32) for g in range(NG)]
              for t in range(T - 1)]
    ccg_in = nc.dram_tensor("ccg_in", [H, 1], f32)
    ccg_out = nc.dram_tensor("ccg_out", [H, 1], f32)
    groups = [[0, 1], [2, 3], [4, 5], [6, 7]]

    e_dt = bf16
    rows_per = NLOC // CH  # 16

    with tile.TileContext(nc) as tc:
        import contextlib

        with contextlib.ExitStack() as ctx:
            singles = ctx.enter_context(tc.tile_pool(name="singles", bufs=1))
            work = ctx.enter_context(tc.tile_pool(name="work", bufs=3))
            eloop = ctx.enter_context(tc.tile_pool(name="eloop", bufs=8))
            psp = ctx.enter_context(tc.tile_pool(name="psp", bufs=6, space="PSUM"))

            # ---------------- weights/constants to SBUF ----------------
            # critical path order: preprocess deps, adjacency chunks (spread
            # over 4 DMA queues), message weights, then late (GRU/readout).
            xT_sb = work.tile([F, N], f32, name="xT_sb", tag="xT_sb")
            nc.sync.dma_start(out=xT_sb[:], in_=xT_d[:])
            Wpre1_sb = singles.tile([F, H], f32)
            nc.sync.dma_start(out=Wpre1_sb[:], in_=w_pre1[:])

            def _load(shape, dram, name, q=nc.sync):
                t_ = singles.tile(list(shape), f32, name=name, tag=name)
                q.dma_start(out=t_[:], in_=dram[:])
                return t_

            preb1 = _load([128, HT], preb1_d, "preb1")
            preb2 = _load([128, HT], preb2_d, "preb2")
            msgb1 = _load([128, HT], msgb1_d, "msgb1")
            W_pre2 = _WSb(nc, singles, w_pre2[:], H, H, "Wpre2")

            # adjacency rows (bf16) + broadcast across partitions
            adj_sb = singles.tile([NLOC, N], bf16)
            nc.scalar.dma_start(out=adj_sb[:], in_=adj_d[:])
            adj_bct = [
                singles.tile([128, rows_per * N], e_dt, name=f"adjbc{c}", tag=f"adjbc{c}")
                for c in range(CH)
            ]
            bc_q = [nc.sync, nc.gpsimd, nc.scalar]
            for c in range(CH):
                bc_in = bass.AP(
                    tensor=adj_d,
                    offset=c * rows_per * N,
                    ap=[[0, 128], [1, rows_per * N]],
                )
                bc_q[c % 3].dma_start(out=adj_bct[c][:], in_=bc_in)

            W_m1j = _WSb(nc, singles, w_m1j[:], H, H, "Wm1j")
            W_m1i = _WSb(nc, singles, w_m1i[:], H, H, "Wm1i")
            W_m1jn = _WSb(nc, singles, w_m1jn[:], H, H, "Wm1jn")
            W_m2 = _WSb(nc, singles, w_m2[:], H, H, "Wm2", queue=nc.gpsimd)
            W_ih = _WSb(nc, singles, w_ih[:], H, 3 * H, "Wih", queue=nc.gpsimd)
            W_hh = _WSb(nc, singles, w_hh[:], H, 3 * H, "Whh", queue=nc.gpsimd)
            W_ro1 = _WSb(nc, singles, w_ro1[:], H, H, "Wro1", queue=nc.scalar)
            W_ro2 = _WSb(nc, singles, w_ro2[:], H, A, "Wro2", queue=nc.scalar)

            msgb2 = _load([1, H], msgb2_d, "msgb2", q=nc.gpsimd)
            brz = _load([128, 4], brz_d, "brz", q=nc.gpsimd)
            bihn = _load([128, HT], bihn_d, "bihn", q=nc.gpsimd)
            bhhn = _load([128, HT], bhhn_d, "bhhn", q=nc.gpsimd)
            rob1 = _load([128, HT], rob1_d, "rob1", q=nc.scalar)
            rob2 = _load([A, 1], rob2_d, "rob2", q=nc.scalar)
            ident = _load([128, 128], ident_d, "ident")

            # degree: adj_sb holds 32*(adj-1) -> deg = reduce/32 + N
            deg_col = singles.tile([NLOC, 1], f32)
            nc.vector.reduce_sum(deg_col[:], adj_sb[:], axis=mybir.AxisListType.X)
            nc.vector.tensor_scalar(
                deg_col[:], deg_col[:], 1.0 / 32.0, float(N),
                mybir.AluOpType.mult, mybir.AluOpType.add,
            )
            ps_t = psp.tile([128, 512], f32, name="ps", tag="ps")
            nc.tensor.transpose(ps_t[0:1, 0:NLOC], deg_col[:], ident[:])
            deg_row = singles.tile([1, NLOC], f32)
            nc.vector.tensor_copy(deg_row[:], ps_t[0:1, 0:NLOC])

            # ---------------- preprocess: h0 (full width: local+partner) ----
            h0 = [singles.tile([128, N], f32, name=f"h0_{ht}", tag=f"h0_{ht}") for ht in range(HT)]
            p1 = [work.tile([128, N], f32, name=f"p1_{ht}", tag=f"p1_{ht}") for ht in range(HT)]
            for ht in range(HT):
                ps = psp.tile([128, 512], f32, name="ps", tag="ps")
                nc.tensor.matmul(
                    ps[:, 0:N], Wpre1_sb[:, ht * 128:(ht + 1) * 128], xT_sb[:],
                    start=True, stop=True,
                )
                nc.scalar.activation(
                    p1[ht][:], ps[:, 0:N], mybir.ActivationFunctionType.Relu,
                    bias=preb1[:, ht:ht + 1],
                )
            for ht in range(HT):
                ps = psp.tile([128, 512], f32, name="ps", tag="ps")
                _mm_acc(nc, ps[:, 0:N], W_pre2, ht * 128, p1, HT)
                nc.scalar.activation(
                    h0[ht][:], ps[:, 0:N], mybir.ActivationFunctionType.Identity,
                    bias=preb2[:, ht:ht + 1],
                )

            # hjbT per-iteration ring tiles [128, N] bf16 (b1 folded in)
            hjbT = [None, None]
            # iteration 0: full-width hjbT/hiT from h0
            for ht in range(HT):
                ps = psp.tile([128, 512], f32, name="ps", tag="ps")
                _mm_acc(nc, ps[:, 0:N], W_m1j, ht * 128, h0, HT)
                t_ = work.tile([128, N], e_dt, name=f"hjbT{ht}", tag=f"hjbT{ht}")
                nc.scalar.activation(
                    t_[:], ps[:, 0:N], mybir.ActivationFunctionType.Identity,
                    bias=msgb1[:, ht:ht + 1],
                )
                hjbT[ht] = t_
            hiTf = [None, None]
            h0loc = [h_[:, 0:NLOC] for h_ in h0]
            for ht in range(HT):
                ps = psp.tile([128, 512], f32, name="ps", tag="ps")
                _mm_acc(nc, ps[:, 0:NLOC], W_m1i, ht * 128, h0loc, HT)
                t_ = work.tile([128, NLOC], f32, name=f"hiTf{ht}", tag=f"hiTf{ht}")
                nc.vector.tensor_copy(t_[:], ps[:, 0:NLOC])
                hiTf[ht] = t_

            # h state: local columns only, ht-concatenated [128, HT*NLOC]
            def h_slice(hcat, ht, c0, c1):
                return hcat[:, ht * NLOC + c0: ht * NLOC + c1]

            h_cur = None  # t=0 uses h0loc directly

            # ---------------- message passing iterations ----------------
            for t in range(T):
                last = (t == T - 1)
                aggT = [work.tile([128, NLOC], f32, name=f"aggT{ht}", tag=f"aggT{ht}")
                        for ht in range(HT)]
                hcat_new = work.tile([128, HT * NLOC], f32, name="hcat", tag="hcat")

                hjb_rep = [
                    bass.AP(
                        tensor=hjbT[ht].tensor, offset=hjbT[ht].offset,
                        ap=[hjbT[ht].ap[0], [0, SC_OF_16], [1, N]],
                    )
                    for ht in range(HT)
                ]

                def run_block(blk):
                    g0 = blk * rows_per
                    ch = adj_bct[blk]
                    ws = []
                    for ht in range(HT):
                        if SC_OF_16 > 0:
                            w = eloop.tile([128, SC_OF_16 * N], e_dt,
                                           name=f"w{ht}", tag=f"w{ht}")
                            eng = nc.gpsimd if ht in MASK_GPS_HT else nc.vector
                            eng.tensor_tensor(
                                out=w[:], in0=hjb_rep[ht],
                                in1=ch[:, 0:SC_OF_16 * N],
                                op=mybir.AluOpType.add,
                            )
                            ws.append(w)
                        else:
                            ws.append(None)
                    for k in range(16):
                        i = g0 + k
                        for ht in range(HT):
                            scr = eloop.tile([128, N], e_dt, name="scr", tag="scr")
                            if k < SC_OF_16:
                                nc.scalar.activation(
                                    scr[:], ws[ht][:, k * N:(k + 1) * N],
                                    mybir.ActivationFunctionType.Relu,
                                    bias=hiTf[ht][:, i:i + 1],
                                    accum_out=aggT[ht][:, i:i + 1],
                                )
                            else:
                                nc.vector._custom_dve(
                                    MSG_RELU_ACC,
                                    out=scr[:], in0=hjbT[ht][:],
                                    in1=ch[:, k * N:(k + 1) * N],
                                    s0=hiTf[ht][:, i:i + 1], s1=0.0,
                                    accum_out=aggT[ht][:, i:i + 1],
                                )

                def group_tail(g):
                    c0, c1 = g * GW, (g + 1) * GW
                    # msgT columns for this group
                    msgT = []
                    for ht in range(HT):
                        ps = psp.tile([128, 512], f32, name="ps", tag="ps")
                        _mm_acc(
                            nc, ps[:, 0:GW], W_m2, ht * 128,
                            [a_[:, c0:c1] for a_ in aggT], HT,
                            extra=(msgb2[0:1, ht * 128:(ht + 1) * 128],
                                   deg_row[:, c0:c1]),
                        )
                        m_ = work.tile([128, GW], f32, name=f"msgT{ht}_{g}",
                                       tag=f"msgT{ht}_{g}")
                        nc.vector.tensor_copy(m_[:], ps[:, 0:GW])
                        msgT.append(m_)

                    # GRU gates (torch order r, z, n), columns c0:c1
                    if h_cur is None:
                        hloc = [h0[ht][:, c0:c1] for ht in range(HT)]
                    else:
                        hloc = [h_slice(h_cur, ht, c0, c1) for ht in range(HT)]
                    ps_rz = psp.tile([128, 512], f32, name="ps", tag="ps")
                    for mt in range(4):  # r0 r1 z0 z1
                        for kt in range(HT):
                            nc.tensor.matmul(
                                ps_rz[:, mt * GW:(mt + 1) * GW],
                                W_ih[:, kt * 768 + mt * 128: kt * 768 + (mt + 1) * 128],
                                msgT[kt][:], start=(kt == 0), stop=False,
                            )
                        for kt in range(HT):
                            nc.tensor.matmul(
                                ps_rz[:, mt * GW:(mt + 1) * GW],
                                W_hh[:, kt * 768 + mt * 128: kt * 768 + (mt + 1) * 128],
                                hloc[kt], start=False, stop=(kt == HT - 1),
                            )
                    rz = work.tile([128, 4 * GW], f32, name=f"rz{g}", tag=f"rz{g}")
                    for mt in range(4):
                        nc.scalar.activation(
                            rz[:, mt * GW:(mt + 1) * GW],
                            ps_rz[:, mt * GW:(mt + 1) * GW],
                            mybir.ActivationFunctionType.Sigmoid,
                            bias=brz[:, mt:mt + 1],
                        )
                    # gi_n and gh_n accumulate in SEPARATE psum tiles: interleaved
                    # start/stop accumulation groups sharing one PSUM bank
                    # clobber each other.
                    ps_gin = psp.tile([128, 512], f32, name="ps", tag="ps")
                    ps_ghn = psp.tile([128, 512], f32, name="ps", tag="ps")
                    for ht in range(HT):
                        for kt in range(HT):
                            nc.tensor.matmul(
                                ps_gin[:, ht * GW:(ht + 1) * GW],
                                W_ih[:, kt * 768 + (4 + ht) * 128: kt * 768 + (5 + ht) * 128],
                                msgT[kt][:], start=(kt == 0), stop=(kt == HT - 1),
                            )
                            nc.tensor.matmul(
                                ps_ghn[:, ht * GW:(ht + 1) * GW],
                                W_hh[:, kt * 768 + (4 + ht) * 128: kt * 768 + (5 + ht) * 128],
                                hloc[kt], start=(kt == 0), stop=(kt == HT - 1),
                            )
                    for ht in range(HT):
                        hn = work.tile([128, GW], f32, name="hn", tag="hn")
                        nc.scalar.activation(
                            hn[:], ps_ghn[:, ht * GW:(ht + 1) * GW],
                            mybir.ActivationFunctionType.Identity,
                            bias=bhhn[:, ht:ht + 1],
                        )
                        rhn = work.tile([128, GW], f32, name="rhn", tag="rhn")
                        nc.vector.tensor_mul(rhn[:], rz[:, ht * GW:(ht + 1) * GW], hn[:])
                        nsum = work.tile([128, GW], f32, name="nsum", tag="nsum")
                        nc.vector.tensor_add(
                            nsum[:], rhn[:], ps_gin[:, ht * GW:(ht + 1) * GW]
                        )
                        n_t = work.tile([128, GW], f32, name="n_t", tag="n_t")
                        nc.scalar.activation(
                            n_t[:], nsum[:], mybir.ActivationFunctionType.Tanh,
                            bias=bihn[:, ht:ht + 1],
                        )
                        # h' = n + z*(h - n)
                        hmn = work.tile([128, GW], f32, name="hmn", tag="hmn")
                        nc.vector.tensor_sub(hmn[:], hloc[ht], n_t[:])
                        zh = work.tile([128, GW], f32, name="zh", tag="zh")
                        nc.vector.tensor_mul(
                            zh[:], rz[:, (2 + ht) * GW: (3 + ht) * GW], hmn[:]
                        )
                        nc.vector.tensor_add(
                            h_slice(hcat_new, ht, c0, c1), n_t[:], zh[:]
                        )

                # ---- emit: e-loop group 0, tail(0)+cc, e-loop group 1, ... --
                for blk in range(0, CH // NG):
                    run_block(blk)
                group_tail(0)
                if not last:
                    for ht in range(HT):
                        nc.sync.dma_start(
                            out=cc_in[t][0][ht * 128:(ht + 1) * 128, :],
                            in_=h_slice(hcat_new, ht, 0, GW),
                        )
                    nc.gpsimd.collective_compute(
                        "AllReduce", mybir.AluOpType.add, replica_groups=groups,
                        ins=[cc_in[t][0][:]], outs=[cc_out[t][0][:]],
                    )
                for blk in range(CH // NG, CH):
                    run_block(blk)
                group_tail(1)
                if not last:
                    for ht in range(HT):
                        nc.sync.dma_start(
                            out=cc_in[t][1][ht * 128:(ht + 1) * 128, :],
                            in_=h_slice(hcat_new, ht, GW, NLOC),
                        )
                    nc.gpsimd.collective_compute(
                        "AllReduce", mybir.AluOpType.add, replica_groups=groups,
                        ins=[cc_in[t][1][:]], outs=[cc_out[t][1][:]],
                    )
                    # ---- next-iteration hiT + local hjbT (overlap the cc) --
                    hnewloc = [
                        bass.AP(
                            tensor=hcat_new.tensor,
                            offset=hcat_new.offset + ht * NLOC,
                            ap=[hcat_new.ap[0], [1, NLOC]],
                        )
                        for ht in range(HT)
                    ]
                    for ht in range(HT):
                        ps = psp.tile([128, 512], f32, name="ps", tag="ps")
                        _mm_acc(nc, ps[:, 0:NLOC], W_m1i, ht * 128, hnewloc, HT)
                        t_ = work.tile([128, NLOC], f32, name=f"hiTf{ht}", tag=f"hiTf{ht}")
                        nc.vector.tensor_copy(t_[:], ps[:, 0:NLOC])
                        hiTf[ht] = t_
                    for ht in range(HT):
                        ps = psp.tile([128, 512], f32, name="ps", tag="ps")
                        _mm_acc(nc, ps[:, 0:NLOC], W_m1j, ht * 128, hnewloc, HT)
                        t_ = work.tile([128, N], e_dt, name=f"hjbT{ht}", tag=f"hjbT{ht}")
                        nc.scalar.activation(
                            t_[:, 0:NLOC], ps[:, 0:NLOC],
                            mybir.ActivationFunctionType.Identity,
                            bias=msgb1[:, ht:ht + 1],
                        )
                        hjbT[ht] = t_
                    # ---- partner hjbT columns from each group's AllReduce --
                    for g in range(NG):
                        c0 = g * GW
                        rem = work.tile([128, HT * GW], f32, name=f"rem{g}", tag=f"rem{g}")
                        for ht in range(HT):
                            nc.sync.dma_start(
                                out=rem[:, ht * GW:(ht + 1) * GW],
                                in_=cc_out[t][g][ht * 128:(ht + 1) * 128, :],
                            )
                        rem_t = [rem[:, ht * GW:(ht + 1) * GW] for ht in range(HT)]
                        hnew_t = [h_slice(hcat_new, ht, c0, c0 + GW) for ht in range(HT)]
                        for ht in range(HT):
                            ps = psp.tile([128, 512], f32, name="ps", tag="ps")
                            for kt in range(HT):
                                nc.tensor.matmul(
                                    ps[:, 0:GW],
                                    W_m1j[:, kt * H + ht * 128: kt * H + ht * 128 + 128],
                                    rem_t[kt], start=(kt == 0), stop=False,
                                )
                            for kt in range(HT):
                                nc.tensor.matmul(
                                    ps[:, 0:GW],
                                    W_m1jn[:, kt * H + ht * 128: kt * H + ht * 128 + 128],
                                    hnew_t[kt], start=False, stop=(kt == HT - 1),
                                )
                            nc.scalar.activation(
                                hjbT[ht][:, NLOC + c0: NLOC + c0 + GW], ps[:, 0:GW],
                                mybir.ActivationFunctionType.Identity,
                                bias=msgb1[:, ht:ht + 1],
                            )
                h_cur = hcat_new

            # ---------------- readout: g = sum_nodes h (pairwise cc) --------
            gT = [work.tile([128, 1], f32, name=f"gT{ht}", tag=f"gT{ht}") for ht in range(HT)]
            for ht in range(HT):
                nc.vector.reduce_sum(
                    gT[ht][:], h_slice(h_cur, ht, 0, NLOC), axis=mybir.AxisListType.X
                )
                nc.sync.dma_start(out=ccg_in[ht * 128:(ht + 1) * 128, :], in_=gT[ht][:])
            nc.gpsimd.collective_compute(
                "AllReduce", mybir.AluOpType.add, replica_groups=groups,
                ins=[ccg_in[:]], outs=[ccg_out[:]],
            )
            for ht in range(HT):
                nc.sync.dma_start(out=gT[ht][:], in_=ccg_out[ht * 128:(ht + 1) * 128, :])
            y1 = [work.tile([128, 1], f32, name=f"y1{ht}", tag=f"y1{ht}") for ht in range(HT)]
            for ht in range(HT):
                ps = psp.tile([128, 512], f32, name="ps", tag="ps")
                _mm_acc(nc, ps[:, 0:1], W_ro1, ht * 128, gT, HT)
                nc.scalar.activation(
                    y1[ht][:], ps[:, 0:1], mybir.ActivationFunctionType.Relu,
                    bias=rob1[:, ht:ht + 1],
                )
            ps_q = psp.tile([128, 512], f32, name="ps", tag="ps")
            for kt in range(HT):
                nc.tensor.matmul(
                    ps_q[0:A, 0:1], W_ro2[:, kt * A:(kt + 1) * A], y1[kt][:],
                    start=(kt == 0), stop=(kt == HT - 1),
                )
            q_sb = work.tile([A, 1], f32, name="q_sb", tag="q_sb")
            nc.scalar.activation(
                q_sb[:], ps_q[0:A, 0:1], mybir.ActivationFunctionType.Identity,
                bias=rob2[:],
            )
            nc.sync.dma_start(out=q_out[:], in_=q_sb[:])

    nc.compile()
    return nc


def _in_maps(inputs):
    nf = np.asarray(inputs["node_features"], np.float32)
    adj = np.asarray(inputs["adjacency"])
    msg_W1 = np.asarray(inputs["msg_W1"], np.float32)
    gbih = np.asarray(inputs["gru_bih"], np.float32)
    gbhh = np.asarray(inputs["gru_bhh"], np.float32)

    def cols(v, nt):  # [nt*128] -> [128, nt] partition-major columns
        return np.ascontiguousarray(np.asarray(v, np.float32).reshape(nt, 128).T)

    shared = {
        "pre_W1": np.asarray(inputs["pre_W1"], np.float32),
        "pre_W2": np.asarray(inputs["pre_W2"], np.float32),
        "W1i": np.ascontiguousarray(msg_W1[:H]),
        "W1j": np.ascontiguousarray(msg_W1[H:]),
        "W1jn": np.ascontiguousarray(-msg_W1[H:]),
        "W2m": np.asarray(inputs["msg_W2"], np.float32),
        "Wih": np.asarray(inputs["gru_Wih"], np.float32),
        "Whh": np.asarray(inputs["gru_Whh"], np.float32),
        "roW1": np.asarray(inputs["ro_W1"], np.float32),
        "roW2": np.asarray(inputs["ro_W2"], np.float32),
        "preb1c": cols(inputs["pre_b1"], HT),
        "preb2c": cols(inputs["pre_b2"], HT),
        "msgb1c": cols(inputs["msg_b1"], HT),
        "msgb2r": np.asarray(inputs["msg_b2"], np.float32)[None, :],
        "brzc": cols((gbih + gbhh)[: 2 * H], 4),
        "bihnc": cols(gbih[2 * H:], HT),
        "bhhnc": cols(gbhh[2 * H:], HT),
        "rob1c": cols(inputs["ro_b1"], HT),
        "rob2c": np.asarray(inputs["ro_b2"], np.float32)[:, None],
        "ident": np.eye(128, dtype=np.float32),
    }
    maps = []
    for c in range(8):
        b, half = c // 2, c % 2
        lo, hi = half * NLOC, (half + 1) * NLOC
        perm = np.r_[lo:hi, 0:lo, hi:N]
        m = dict(shared)
        m["xT"] = np.ascontiguousarray(nf[b].T[:, perm])
        m["adjb"] = np.ascontiguousarray(
            ((adj[b, lo:hi][:, perm] - 1) * 32).astype(BF16_NP)
        )
        maps.append(m)
    return maps


def kernel(**inputs) -> np.ndarray:
    if "nc" not in _CACHE:
        _CACHE["nc"] = build_program()
    nc = _CACHE["nc"]
    maps = _in_maps(inputs)
    res = run_bass_kernel_spmd(nc, maps, list(range(8))).results
    q = np.stack([res[2 * b]["q_out"][:, 0] for b in range(B)]).astype(np.float32)
    return q
